# revision 4
# baseline (speedup 1.0000x reference)
"""Self-contained Trainium2 Bass kernel for nn_Attention (dense transformer MHA block).

Full inputs in, full outputs out. Sharding: batch (B=8) data-parallel across the
8 NeuronCores -- one batch element per core, weights replicated. No collectives.

Per-core math (x: [1024, 768], H=12 heads, D=64):
  qkv = x @ qkv_w.T ; q,k,v split ; per head: softmax(q k^T / 8) v ; proj + bias.

Layout strategy (all fp32):
  - x^T and W^T produced on-chip via PE transposes (fp32 has no DMA transpose).
  - q^T,k^T computed in [o, i] layout (features on partitions)  -> directly usable
    as S^T = k^T.T @ q^T matmul operands (contraction over d on partitions).
  - v computed in natural [token, feature] layout with an extra ones column; the
    O' = E^T.T @ [v | 1] matmul then yields both the attention output AND the
    softmax row-sums in one pass (column 65 trick), avoiding any partition-axis
    reduction and any transpose of the attention matrix.
  - softmax without max-subtraction (scores are ~N(0,1); fp32 exp is safe).
  - normalization: approx reciprocal (2 ULP) of the rowsum row + PE ones-matmul
    partition-broadcast + one tensor_tensor multiply per head.
  - attnout lands directly in [feature, token] layout = proj's lhsT; proj output
    is natural [token, feature] and DMAs straight out.
"""

import os
import sys

for _p in ("/opt/trn_rl_repo",):
    if os.path.isdir(_p) and _p not in sys.path:
        sys.path.insert(0, _p)

import numpy as np

P = 128
N = 1024          # tokens per batch element
C = 768           # model dim
H = 12            # heads
D = 64            # head dim
B = 8             # batch (== n cores)
NB = N // P       # 8 token blocks
CB = C // P       # 6 feature blocks
SCALE = D ** -0.5  # 0.125
F32 = None         # set after import


def build_attention_bass():
    import concourse.mybir as mybir
    import concourse.tile as tile
    from concourse import bacc
    from concourse.masks import make_identity

    f32 = mybir.dt.float32
    nc = bacc.Bacc("TRN2", target_bir_lowering=False, debug=False)

    x = nc.dram_tensor("x", [N, C], f32, kind="ExternalInput")
    qkv_w = nc.dram_tensor("qkv_w", [3 * C, C], f32, kind="ExternalInput")
    proj_w = nc.dram_tensor("proj_w", [C, C], f32, kind="ExternalInput")
    proj_b = nc.dram_tensor("proj_b", [C], f32, kind="ExternalInput")
    out = nc.dram_tensor("out", [N, C], f32, kind="ExternalOutput")

    x_r = x.rearrange("(nb p) c -> nb p c", p=P)       # [8, 128, 768]
    w_r = qkv_w.rearrange("(ob p) c -> ob p c", p=P)   # [18, 128, 768]
    pw_r = proj_w.rearrange("(ob p) c -> ob p c", p=P)  # [6, 128, 768]
    out_r = out.rearrange("(nb p) c -> nb p c", p=P)

    with tile.TileContext(nc) as tc:
        with tc.tile_pool(name="persist", bufs=1) as pA, \
             tc.tile_pool(name="span", bufs=1) as pB1:
            # ---- long-lived tensors
            qkT = pA.tile([P, 2 * CB, N], f32)         # q^T & k^T: [o%128, ob, i]
            vext = pA.tile([P, NB, H, D + 1], f32)     # v natural + ones col
            ident = pA.tile([P, P], f32)
            ones_row = pA.tile([1, P], f32)
            attnoutT = pB1.tile([P, CB, N], f32)       # [c2%128, cb2, i]
            pwT = pB1.tile([P, CB, C], f32)            # proj_w^T [c, cb, o2]
            bias_bc = pB1.tile([P, C], f32)            # proj_b broadcast

            make_identity(nc, ident[:])
            nc.vector.memset(ones_row[:], 1.0)
            nc.gpsimd.memset(vext[:, :, :, D:D + 1], 1.0)
            nc.gpsimd.dma_start(bias_bc[:], proj_b[None, :].to_broadcast((P, C)))

            # ================= phase 0+1: x^T, W^T, qkv =================
            with tc.tile_pool(name="ph1sb", bufs=1) as p_big, \
                 tc.tile_pool(name="ph1roll", bufs=4) as p_roll, \
                 tc.tile_pool(name="ph1w", bufs=3) as p_w:
                xT = p_big.tile([P, CB, N], f32)        # x^T: [c%128, cb, i]
                wTv = p_big.tile([P, CB, C], f32)       # v-part of qkv_w^T

                # ---- x -> x^T (48 PE transposes, grouped 4 per psum tile)
                with tc.tile_pool(name="tpsx", bufs=6, space="PSUM") as tpsx, \
                     nc.named_scope("x_transpose"):
                    for nbg in range(2):
                        xnat = []
                        for j in range(4):
                            t = p_roll.tile([P, C], f32, tag="xnat")
                            nc.sync.dma_start(t[:], x_r[nbg * 4 + j])
                            xnat.append(t)
                        for cb in range(CB):
                            pst = tpsx.tile([P, 512], f32, tag="tpsx")
                            for j in range(4):
                                nc.tensor.transpose(
                                    pst[:, j * P:(j + 1) * P],
                                    xnat[j][:, cb * P:(cb + 1) * P],
                                    ident[:])
                            nc.any.tensor_copy(
                                xT[:, cb, nbg * 512:(nbg + 1) * 512], pst[:])

                with tc.tile_pool(name="tpsw", bufs=2, space="PSUM") as tpsw, \
                     tc.tile_pool(name="ps1", bufs=2, space="PSUM") as ps1p:
                    # ---- q^T, k^T : for each of 12 o-blocks
                    with nc.named_scope("qkT"):
                        for ob in range(2 * CB):
                            wnat = p_w.tile([P, C], f32, tag="wnat")
                            nc.sync.dma_start(wnat[:], w_r[ob])
                            wT = p_w.tile([P, CB, P], f32, tag="wTqk")
                            pst = tpsw.tile([P, C], f32, tag="tpsw")
                            for cb in range(CB):
                                nc.tensor.transpose(
                                    pst[:, cb * P:(cb + 1) * P],
                                    wnat[:, cb * P:(cb + 1) * P], ident[:])
                            nc.any.tensor_copy(
                                wT[:], pst[:].rearrange("p (cb k) -> p cb k", cb=CB))
                            for ic in range(2):
                                ps1 = ps1p.tile([P, 512], f32, tag="ps1")
                                for cb in range(CB):
                                    nc.tensor.matmul(
                                        ps1[:], wT[:, cb, :],
                                        xT[:, cb, ic * 512:(ic + 1) * 512],
                                        start=(cb == 0), stop=(cb == CB - 1))
                                nc.any.tensor_copy(
                                    qkT[:, ob, ic * 512:(ic + 1) * 512], ps1[:])

                    # ---- v-part of W^T
                    with nc.named_scope("wTv"):
                        for obv in range(CB):
                            wnat = p_w.tile([P, C], f32, tag="wnat")
                            nc.sync.dma_start(wnat[:], w_r[2 * CB + obv])
                            pst = tpsw.tile([P, C], f32, tag="tpsw")
                            for cb in range(CB):
                                nc.tensor.transpose(
                                    pst[:, cb * P:(cb + 1) * P],
                                    wnat[:, cb * P:(cb + 1) * P], ident[:])
                            nc.any.tensor_copy(
                                wTv[:, :, obv * P:(obv + 1) * P],
                                pst[:].rearrange("p (cb k) -> p cb k", cb=CB))

                    # ---- proj_w^T (needed in phase 3; built here while psum free)
                    with nc.named_scope("pwT"):
                        for obp in range(CB):
                            wnat = p_w.tile([P, C], f32, tag="wnat")
                            nc.sync.dma_start(wnat[:], pw_r[obp])
                            pst = tpsw.tile([P, C], f32, tag="tpsw")
                            for cb in range(CB):
                                nc.tensor.transpose(
                                    pst[:, cb * P:(cb + 1) * P],
                                    wnat[:, cb * P:(cb + 1) * P], ident[:])
                            nc.any.tensor_copy(
                                pwT[:, :, obp * P:(obp + 1) * P],
                                pst[:].rearrange("p (cb k) -> p cb k", cb=CB))

                    # ---- v (natural layout) into vext
                    with nc.named_scope("v"):
                        for jb in range(NB):
                            ps2 = tpsw.tile([P, C], f32, tag="tpsw")
                            for (o0, w) in ((0, 512), (512, 256)):
                                for cb in range(CB):
                                    nc.tensor.matmul(
                                        ps2[:, o0:o0 + w],
                                        xT[:, cb, jb * P:(jb + 1) * P],
                                        wTv[:, cb, o0:o0 + w],
                                        start=(cb == 0), stop=(cb == CB - 1))
                            nc.any.tensor_copy(
                                vext[:, jb, :, 0:D],
                                ps2[:].rearrange("p (h d) -> p h d", h=H))

            # ================= phase 2: attention per head =================
            with tc.tile_pool(name="ph2sb", bufs=3) as p_et, \
                 tc.tile_pool(name="ph2sm", bufs=2) as p_sm, \
                 tc.tile_pool(name="pss", bufs=2, space="PSUM") as pssp, \
                 tc.tile_pool(name="pso", bufs=1, space="PSUM") as psop, \
                 tc.tile_pool(name="psr", bufs=1, space="PSUM") as psrp, \
                 nc.named_scope("attention"):
                for h in range(H):
                    hp, hb = h % 2, h // 2
                    r0, r1 = hp * D, hp * D + D
                    # S^T = k^T.T @ q^T ; E^T = exp(S^T/8)
                    ets = []
                    for jbg in range(2):
                        et = p_et.tile([P, 4, N], f32, tag="et")
                        ets.append(et)
                        for jj in range(4):
                            jb = jbg * 4 + jj
                            ps_s = pssp.tile([P, N], f32, tag="pss")
                            for ic in range(2):
                                nc.tensor.matmul(
                                    ps_s[:, ic * 512:(ic + 1) * 512],
                                    qkT[r0:r1, CB + hb, jb * P:(jb + 1) * P],
                                    qkT[r0:r1, hb, ic * 512:(ic + 1) * 512],
                                    start=True, stop=True)
                            nc.scalar.activation(
                                et[:, jj, :], ps_s[:],
                                mybir.ActivationFunctionType.Exp, scale=SCALE)
                    # O'^T = [v|1].T @ E^T  (rows 0..63 = out, row 64 = rowsum)
                    ps_o = psop.tile([D + 1, N], f32, tag="pso")
                    for jb in range(NB):
                        for ic in range(2):
                            nc.tensor.matmul(
                                ps_o[:, ic * 512:(ic + 1) * 512],
                                vext[:, jb, h, :],
                                ets[jb // 4][:, jb % 4, ic * 512:(ic + 1) * 512],
                                start=(jb == 0), stop=(jb == NB - 1))
                    # normalize: r = 1/rowsum ; broadcast via ones matmul ; mult
                    # (custom-DVE recip reads garbage from PSUM on HW -- stage
                    # the rowsum row through SBUF via ACT first)
                    r = p_sm.tile([1, N], f32, tag="r")
                    rs = p_sm.tile([1, N], f32, tag="rs")
                    scr = p_sm.tile([1, N], f32, tag="scr")
                    nc.scalar.copy(rs[:], ps_o[D:D + 1, :])
                    nc.vector.reciprocal_approx_accurate(r[:], rs[:], scr[:])
                    ps_r = psrp.tile([D, N], f32, tag="psr")
                    for ic in range(2):
                        nc.tensor.matmul(
                            ps_r[:, ic * 512:(ic + 1) * 512],
                            ones_row[:, 0:D], r[:, ic * 512:(ic + 1) * 512],
                            start=True, stop=True)
                    rb = p_sm.tile([D, N], f32, tag="rb")
                    nc.any.tensor_copy(rb[:], ps_r[:])
                    nc.vector.tensor_tensor(
                        attnoutT[r0:r1, hb, :], ps_o[0:D, :], rb[:],
                        mybir.AluOpType.mult)

            # ================= phase 3: proj =================
            with tc.tile_pool(name="ph3sb", bufs=2) as p_osb, \
                 tc.tile_pool(name="ps3", bufs=2, space="PSUM") as ps3p, \
                 nc.named_scope("proj"):
                for nb in range(NB):
                    ps3 = ps3p.tile([P, C], f32, tag="ps3")
                    for (o0, w) in ((0, 512), (512, 256)):
                        for cb in range(CB):
                            nc.tensor.matmul(
                                ps3[:, o0:o0 + w],
                                attnoutT[:, cb, nb * P:(nb + 1) * P],
                                pwT[:, cb, o0:o0 + w],
                                start=(cb == 0), stop=(cb == CB - 1))
                    osb = p_osb.tile([P, C], f32, tag="osb")
                    nc.vector.tensor_tensor(
                        osb[:], ps3[:], bias_bc[:], mybir.AluOpType.add)
                    nc.sync.dma_start(out_r[nb], osb[:])

    nc.finalize()
    return nc


_NC_CACHE = None


def kernel(x, qkv_w, proj_w, proj_b):
    """Full inputs -> full output. x: [8, 1024, 768]."""
    global _NC_CACHE
    from concourse.bass_utils import run_bass_kernel_spmd

    if _NC_CACHE is None:
        _NC_CACHE = build_attention_bass()
    nc = _NC_CACHE

    x = np.ascontiguousarray(np.asarray(x, dtype=np.float32))
    qkv_w = np.ascontiguousarray(np.asarray(qkv_w, dtype=np.float32))
    proj_w = np.ascontiguousarray(np.asarray(proj_w, dtype=np.float32))
    proj_b = np.ascontiguousarray(np.asarray(proj_b, dtype=np.float32))

    in_maps = [
        {"x": x[b], "qkv_w": qkv_w, "proj_w": proj_w, "proj_b": proj_b}
        for b in range(B)
    ]
    res = run_bass_kernel_spmd(nc, in_maps, core_ids=list(range(B)))
    return np.stack([res.results[b]["out"] for b in range(B)], axis=0)
